# revision 2
# baseline (speedup 1.0000x reference)
"""Conv4d: F(4,3)^2 Winograd on (V,W), host transforms + Toeplitz-H GEMM.

Per core (8 cores = batch2 x U/4, 6 output-u each):
  - Host: pad, H-window pack (K = 16ci x 8hi = 128), F(4,3) B^T along BOTH
    V and W (6x6 domain planes, 6 vb x 6 wb blocks of 4x4 outputs), cast
    fp16. Device receives the fully transformed tensor:
      bx[(ci,hi)][slab u' (8), plane t (36), (vb6 hb4 wb6)=144]
    packed plane-contiguous so a matmul rhs is [p, (2 slabs), (144)].
  - TensorE: u-pairs. For (p, q, jj, du): one matmul N=288 covers u=2p and
    u=2p+1 (slabs 2p+du, 2p+du+1 stacked in the free dim), K=128, M=96,
    PSUM-accumulate 3 du taps into a [96, 4x512] bank-aligned tile.
    324 matmuls total (93312 rows streamed).
  - ScalarE drains each quad; planes with a high-|A^T| dimension
    (jv or jw in {3,4}) drain to fp32 (20 of 36), the rest to fp16.
  - Host: inverse A^T x A^T, bias, unshuffle.
"""

import sys

if "/opt/trn_rl_repo" not in sys.path:
    sys.path.insert(0, "/opt/trn_rl_repo")

import numpy as np

import concourse.bass as bass
import concourse.mybir as mybir
import concourse.tile as tile
from concourse import bacc
from concourse.bass_utils import run_bass_kernel_spmd

C = 16
KS = 3
S = 24
SP = S + 2
UCORE = 6
USLAB = UCORE + 2
HB = 4               # h blocks
BH = 6               # h outputs per block
HI = 8               # h window size
VB = 6               # v blocks (F(4,3): 4 outputs each)
WB = 6               # w blocks
NJ4 = 6              # F(4,3) domain size per dim
NPL = NJ4 * NJ4      # 36 planes
K_IN = C * HI        # 128
M_OUT = C * BH       # 96
N_CORES = 8
PLF = VB * HB * WB           # 144 cols per (slab, plane)
SLABF = NPL * PLF            # 5184 per slab
NN = 2 * PLF                 # 288 matmul free size (u-pair)
GROUP = 4 * NN               # 1152
NUP = UCORE // 2             # 3 u-pairs
NQ = NPL // 4                # 9 quads
NGRP = NUP * NQ              # 27 groups
OUT_FREE = NGRP * GROUP      # 31104

BT4 = np.array([
    [4, 0, -5, 0, 1, 0],
    [0, -4, -4, 1, 1, 0],
    [0, 4, -4, -1, 1, 0],
    [0, -2, -1, 2, 1, 0],
    [0, 2, -1, -2, 1, 0],
    [0, 4, 0, -5, 0, 1]], dtype=np.float64)
G4 = np.array([
    [1 / 4, 0, 0],
    [-1 / 6, -1 / 6, -1 / 6],
    [-1 / 6, 1 / 6, -1 / 6],
    [1 / 24, 1 / 12, 1 / 6],
    [1 / 24, -1 / 12, 1 / 6],
    [0, 0, 1]], dtype=np.float64)
AT4 = np.array([
    [1, 1, 1, 1, 1, 0],
    [0, 1, -1, 2, -2, 0],
    [0, 1, 1, 4, 4, 0],
    [0, 1, -1, 8, -8, 1]], dtype=np.float64)

# plane order: the 20 high-amplification planes (jv or jw in {3,4}) first —
# they drain to fp32 and their heavier output DMA should not land in the
# tail — then the 16 low planes (fp16 drain)
_HISET = (3, 4)
PLANES = ([(a, b) for a in range(NJ4) for b in range(NJ4)
           if a in _HISET or b in _HISET] +
          [(a, b) for a in range(NJ4) for b in range(NJ4)
           if a not in _HISET and b not in _HISET])
NQ32 = 5             # quads 0-4 drain fp32; quads 5-8 drain fp16
OUT32_FREE = NUP * NQ32 * GROUP          # fp32 output columns
OUT16_FREE = NUP * (NQ - NQ32) * GROUP   # fp16 output columns

_cache = {}


def _build_nc():
    if "nc" in _cache:
        return _cache["nc"]
    f16 = mybir.dt.float16
    f32 = mybir.dt.float32
    nc = bacc.Bacc("TRN2", target_bir_lowering=False, debug=False,
                   num_devices=N_CORES)
    x_dram = nc.dram_tensor("x", [K_IN, USLAB * SLABF], f16,
                            kind="ExternalInput")
    w_dram = nc.dram_tensor("w", [K_IN, NPL * KS * M_OUT], f16,
                            kind="ExternalInput")
    o16_dram = nc.dram_tensor("o16", [M_OUT, OUT16_FREE], f16,
                              kind="ExternalOutput")
    o32_dram = nc.dram_tensor("o32", [M_OUT, OUT32_FREE], f32,
                              kind="ExternalOutput")

    with tile.TileContext(nc) as tc:
        with (
            tc.tile_pool(name="wp", bufs=1) as wp,
            tc.tile_pool(name="bxp", bufs=1) as bxp,
            tc.tile_pool(name="o16p", bufs=6) as o16p,
            tc.tile_pool(name="o32p", bufs=6) as o32p,
            tc.tile_pool(name="ps", bufs=2, space="PSUM") as ps,
        ):
            wt = wp.tile([K_IN, NPL * KS * M_OUT], f16)
            bxt = bxp.tile([K_IN, USLAB * SLABF], f16)

            def dma_slab(k):
                nc.sync.dma_start(bxt[:, k * SLABF:(k + 1) * SLABF],
                                  x_dram[:, k * SLABF:(k + 1) * SLABF])

            wcols = NPL * KS * M_OUT // 8

            def dma_wchunk(q):
                nc.sync.dma_start(wt[:, q * wcols:(q + 1) * wcols],
                                  w_dram[:, q * wcols:(q + 1) * wcols])

            # arrival order: what p=0 consumes first
            dma_slab(0)
            dma_slab(1)
            dma_wchunk(0)
            dma_slab(2)
            dma_slab(3)
            dma_wchunk(1)
            dma_wchunk(2)
            dma_slab(4)
            for q in range(3, 8):
                dma_wchunk(q)
            for k in range(5, USLAB):
                dma_slab(k)

            bv = bxt[:].rearrange("p (s t f) -> p s t f", s=USLAB, t=NPL)

            for p in range(NUP):
                for q in range(NQ):
                    pt = ps.tile([M_OUT, 4 * 512], f32, tag="acc",
                                 name="acc")
                    for du in range(KS):
                        for jj in range(4):
                            t = q * 4 + jj
                            dst = pt[:, jj * 512:jj * 512 + NN]
                            rhs = bv[:, 2 * p + du:2 * p + du + 2, t]
                            col = (t * KS + du) * M_OUT
                            nc.tensor.matmul(
                                dst,
                                wt[:, col:col + M_OUT],
                                rhs,
                                start=(du == 0),
                                stop=(du == KS - 1),
                            )
                    pv = pt[:].rearrange("p (jj x) -> p jj x", jj=4)
                    if q < NQ32:
                        ot = o32p.tile([M_OUT, GROUP], f32, tag="o32",
                                       name="ot32")
                        ov = ot[:].rearrange("p (jj x) -> p jj x", jj=4)
                        nc.scalar.activation(
                            ov, pv[:, :, 0:NN],
                            mybir.ActivationFunctionType.Copy)
                        g = p * NQ32 + q
                        nc.sync.dma_start(
                            o32_dram[:, g * GROUP:(g + 1) * GROUP], ot[:])
                    else:
                        ot = o16p.tile([M_OUT, GROUP], f16, tag="o16",
                                       name="ot16")
                        ov = ot[:].rearrange("p (jj x) -> p jj x", jj=4)
                        nc.scalar.activation(
                            ov, pv[:, :, 0:NN],
                            mybir.ActivationFunctionType.Copy)
                        g = p * (NQ - NQ32) + (q - NQ32)
                        nc.sync.dma_start(
                            o16_dram[:, g * GROUP:(g + 1) * GROUP], ot[:])

    nc.compile()
    _cache["nc"] = nc
    return nc


def _pack_weights(weight):
    w6 = np.asarray(weight, dtype=np.float64).reshape(C, C, KS, KS, KS, KS)
    # gw[jv, jw, co, ci, du, dh]
    gw = np.einsum("av,bw,oiuvhw->aboiuh", G4, G4, w6)
    T = np.zeros((C, HI, NPL, KS, C, BH), dtype=np.float64)
    for i, (jv, jw) in enumerate(PLANES):
        for dh in range(KS):
            for ho in range(BH):
                T[:, ho + dh, i, :, :, ho] = \
                    gw[jv, jw, :, :, :, dh].transpose(1, 2, 0)
    return np.ascontiguousarray(
        T.reshape(K_IN, NPL * KS * M_OUT)).astype(np.float16)


def _pack_input_core(x_n, u0):
    xpad = np.zeros((C, USLAB, SP, SP, SP), dtype=np.float32)
    u_lo = max(0, u0 - 1)
    u_hi = min(S, u0 + UCORE + 1)
    xpad[:, u_lo - (u0 - 1):u_hi - (u0 - 1), 1:S + 1, 1:S + 1, 1:S + 1] = \
        x_n[:, u_lo:u_hi]
    xw = np.empty((USLAB, C, HI, SP, HB, SP), dtype=np.float32)
    xt = xpad.transpose(1, 0, 2, 3, 4)
    for hi in range(HI):
        xw[:, :, hi] = xt[:, :, :, hi::BH, :][:, :, :, :HB, :]
    # fwd V then fwd W, both F(4,3): v' = 4vb+cv, w' = 4wb+cw
    dV = np.stack([xw[:, :, :, cv:cv + 4 * (VB - 1) + 1:4] for cv in range(6)],
                  axis=3)
    xv = np.einsum("jc,uihcvbw->uihjvbw", BT4.astype(np.float32), dV)
    dW = np.stack([xv[..., cw:cw + 4 * (WB - 1) + 1:4] for cw in range(6)],
                  axis=-2)
    bx = np.einsum("kc,uihjvbcw->uihjkvbw", BT4.astype(np.float32), dW)
    # bx[u', ci, hi, jv, jw, vb, hb, wb] -> plane order t, then (vb,hb,wb)
    bx = bx.reshape(USLAB, K_IN, NJ4, NJ4, VB, HB, WB)
    out = np.empty((K_IN, USLAB, NPL, PLF), dtype=np.float16)
    for i, (jv, jw) in enumerate(PLANES):
        out[:, :, i] = bx[:, :, jv, jw].reshape(
            USLAB, K_IN, PLF).transpose(1, 0, 2)
    return np.ascontiguousarray(out.reshape(K_IN, USLAB * SLABF))


def make_in_maps(inputs):
    x = np.asarray(inputs["inputs"], dtype=np.float32)
    w_packed = _pack_weights(inputs["weight"])
    in_maps = []
    for c in range(N_CORES):
        n, u0 = c // 4, (c % 4) * UCORE
        in_maps.append({"x": _pack_input_core(x[n], u0), "w": w_packed})
    return in_maps


def host_finish(results16, results32, bias):
    b = np.asarray(bias, dtype=np.float32).reshape(C)
    at4 = AT4.astype(np.float32)
    full = np.empty((2, C, S, S, S, S), dtype=np.float32)
    for c in range(N_CORES):
        n, u0 = c // 4, (c % 4) * UCORE
        # reassemble domain [co, ho, plane, u, vb, hb, wb]
        d32 = np.asarray(results32[c]).astype(np.float32).reshape(
            M_OUT, NUP, NQ32 * 4, 2, PLF)
        d16 = np.asarray(results16[c]).astype(np.float32).reshape(
            M_OUT, NUP, (NQ - NQ32) * 4, 2, PLF)
        dcat = np.concatenate([d32, d16], axis=2)   # [96, p, t, e, 144]
        dom = np.empty((C, BH, NJ4, NJ4, UCORE, VB, HB, WB), dtype=np.float32)
        dv = dcat.reshape(C, BH, NUP, NPL, 2, VB, HB, WB)
        for i, (jv, jw) in enumerate(PLANES):
            # u = 2p + e
            dom[:, :, jv, jw] = dv[:, :, :, i].reshape(
                C, BH, NUP * 2, VB, HB, WB)
        y = np.einsum("aj,bk,ohjkuvxw->ouxvahwb", at4, at4, dom)
        # y[co, u, hb, vb, a4, ho, wb, b4]
        y = y.reshape(C, UCORE, HB, VB, 4, BH, WB, 4)
        y = y.transpose(0, 1, 3, 4, 2, 5, 6, 7).reshape(C, UCORE, S, S, S)
        full[n, :, u0:u0 + UCORE] = y + b[:, None, None, None, None]
    return full


def kernel(inputs, weight, bias):
    nc = _build_nc()
    in_maps = make_in_maps({"inputs": inputs, "weight": weight})
    res = run_bass_kernel_spmd(nc, in_maps, core_ids=list(range(N_CORES)))
    return host_finish([res.results[c]["o16"] for c in range(N_CORES)],
                       [res.results[c]["o32"] for c in range(N_CORES)],
                       bias)


# revision 3
# speedup vs baseline: 1.0862x; 1.0862x over previous
"""Conv4d: F(4,3)^2 Winograd on (V,W), host transforms + Toeplitz-H GEMM.

Per core (8 cores = batch2 x U/4, 6 output-u each):
  - Host: pad, H-window pack (K = 16ci x 8hi = 128), F(4,3) B^T along BOTH
    V and W (6x6 domain planes, 6 vb x 6 wb blocks of 4x4 outputs), cast
    fp16. Device receives the fully transformed tensor:
      bx[(ci,hi)][slab u' (8), plane t (36), (vb6 hb4 wb6)=144]
    packed plane-contiguous so a matmul rhs is [p, (2 slabs), (144)].
  - TensorE: u-pairs. For (p, q, jj, du): one matmul N=288 covers u=2p and
    u=2p+1 (slabs 2p+du, 2p+du+1 stacked in the free dim), K=128, M=96,
    PSUM-accumulate 3 du taps into a [96, 4x512] bank-aligned tile.
    324 matmuls total (93312 rows streamed).
  - ScalarE drains each quad; planes with a high-|A^T| dimension
    (jv or jw in {3,4}) drain to fp32 (20 of 36), the rest to fp16.
  - Host: inverse A^T x A^T, bias, unshuffle.
"""

import sys

if "/opt/trn_rl_repo" not in sys.path:
    sys.path.insert(0, "/opt/trn_rl_repo")

import numpy as np

import concourse.bass as bass
import concourse.mybir as mybir
import concourse.tile as tile
from concourse import bacc
from concourse.bass_utils import run_bass_kernel_spmd

C = 16
KS = 3
S = 24
SP = S + 2
UCORE = 6
USLAB = UCORE + 2
HB = 4               # h blocks
BH = 6               # h outputs per block
HI = 8               # h window size
VB = 6               # v blocks (F(4,3): 4 outputs each)
WB = 6               # w blocks
NJ4 = 6              # F(4,3) domain size per dim
NPL = NJ4 * NJ4      # 36 planes
K_IN = C * HI        # 128
M_OUT = C * BH       # 96
N_CORES = 8
PLF = VB * HB * WB           # 144 cols per (slab, plane)
SLABF = NPL * PLF            # 5184 per slab
NN = 2 * PLF                 # 288 matmul free size (u-pair)
GROUP = 4 * NN               # 1152
NUP = UCORE // 2             # 3 u-pairs
NQ = NPL // 4                # 9 quads
NGRP = NUP * NQ              # 27 groups
OUT_FREE = NGRP * GROUP      # 31104

BT4 = np.array([
    [4, 0, -5, 0, 1, 0],
    [0, -4, -4, 1, 1, 0],
    [0, 4, -4, -1, 1, 0],
    [0, -2, -1, 2, 1, 0],
    [0, 2, -1, -2, 1, 0],
    [0, 4, 0, -5, 0, 1]], dtype=np.float64)
G4 = np.array([
    [1 / 4, 0, 0],
    [-1 / 6, -1 / 6, -1 / 6],
    [-1 / 6, 1 / 6, -1 / 6],
    [1 / 24, 1 / 12, 1 / 6],
    [1 / 24, -1 / 12, 1 / 6],
    [0, 0, 1]], dtype=np.float64)
AT4 = np.array([
    [1, 1, 1, 1, 1, 0],
    [0, 1, -1, 2, -2, 0],
    [0, 1, 1, 4, 4, 0],
    [0, 1, -1, 8, -8, 1]], dtype=np.float64)

# plane order: the 4 worst-amplification planes (both jv and jw in {3,4})
# first — they drain to fp32 and their heavier output DMA should not land
# in the tail — then the rest (fp16 drain)
_HISET = (3, 4)
PLANES = ([(a, b) for a in range(NJ4) for b in range(NJ4)
           if a in _HISET and b in _HISET] +
          [(a, b) for a in range(NJ4) for b in range(NJ4)
           if not (a in _HISET and b in _HISET)])
NQ32 = 1             # quad 0 drains fp32; quads 1-8 drain fp16
OUT32_FREE = NUP * NQ32 * GROUP          # fp32 output columns
OUT16_FREE = NUP * (NQ - NQ32) * GROUP   # fp16 output columns

_cache = {}


def _build_nc():
    if "nc" in _cache:
        return _cache["nc"]
    f16 = mybir.dt.float16
    f32 = mybir.dt.float32
    nc = bacc.Bacc("TRN2", target_bir_lowering=False, debug=False,
                   num_devices=N_CORES)
    x_dram = nc.dram_tensor("x", [K_IN, USLAB * SLABF], f16,
                            kind="ExternalInput")   # (q9, s8, jj4, 144)
    w_dram = nc.dram_tensor("w", [K_IN, NPL * KS * M_OUT], f16,
                            kind="ExternalInput")
    o16_dram = nc.dram_tensor("o16", [M_OUT, OUT16_FREE], f16,
                              kind="ExternalOutput")
    o32_dram = nc.dram_tensor("o32", [M_OUT, OUT32_FREE], f32,
                              kind="ExternalOutput")

    with tile.TileContext(nc) as tc:
        with (
            tc.tile_pool(name="wp", bufs=1) as wp,
            tc.tile_pool(name="bxp", bufs=1) as bxp,
            tc.tile_pool(name="o16p", bufs=24) as o16p,
            tc.tile_pool(name="o32p", bufs=3) as o32p,
            tc.tile_pool(name="ps", bufs=2, space="PSUM") as ps,
        ):
            wt = wp.tile([K_IN, NPL * KS * M_OUT], f16)
            bxt = bxp.tile([K_IN, USLAB * SLABF], f16)

            SPQ = 2 * 4 * PLF        # slab-pair chunk within a quad
            QF = USLAB * 4 * PLF     # 4608 cols per quad

            def dma_chunk(q, sp):
                # slabs (2sp, 2sp+1) of quad q
                c0 = q * QF + sp * SPQ
                nc.sync.dma_start(bxt[:, c0:c0 + SPQ], x_dram[:, c0:c0 + SPQ])

            wcols = NPL * KS * M_OUT // 9    # one weight chunk per quad

            def dma_wchunk(q):
                nc.sync.dma_start(wt[:, q * wcols:(q + 1) * wcols],
                                  w_dram[:, q * wcols:(q + 1) * wcols])

            # arrival order tracks (p, q) consumption: p=0 needs slabs 0-3
            # of quad q right before its group runs
            for q in range(NQ):
                dma_chunk(q, 0)
                dma_chunk(q, 1)
                dma_wchunk(q)
            for q in range(NQ):
                dma_chunk(q, 2)
                dma_chunk(q, 3)

            bv = bxt[:].rearrange("p (q s j f) -> p q s j f",
                                  q=NQ, s=USLAB, j=4)

            for p in range(NUP):
                for q in range(NQ):
                    pt = ps.tile([M_OUT, 4 * 512], f32, tag="acc",
                                 name="acc")
                    for du in range(KS):
                        for jj in range(4):
                            t = q * 4 + jj
                            dst = pt[:, jj * 512:jj * 512 + NN]
                            rhs = bv[:, q, 2 * p + du:2 * p + du + 2, jj]
                            col = (t * KS + du) * M_OUT
                            nc.tensor.matmul(
                                dst,
                                wt[:, col:col + M_OUT],
                                rhs,
                                start=(du == 0),
                                stop=(du == KS - 1),
                            )
                    pv = pt[:].rearrange("p (jj x) -> p jj x", jj=4)
                    if q < NQ32:
                        ot = o32p.tile([M_OUT, GROUP], f32, tag="o32",
                                       name="ot32")
                        ov = ot[:].rearrange("p (jj x) -> p jj x", jj=4)
                        nc.scalar.activation(
                            ov, pv[:, :, 0:NN],
                            mybir.ActivationFunctionType.Copy)
                        g = p * NQ32 + q
                        nc.sync.dma_start(
                            o32_dram[:, g * GROUP:(g + 1) * GROUP], ot[:])
                    else:
                        ot = o16p.tile([M_OUT, GROUP], f16, tag="o16",
                                       name="ot16")
                        ov = ot[:].rearrange("p (jj x) -> p jj x", jj=4)
                        nc.scalar.activation(
                            ov, pv[:, :, 0:NN],
                            mybir.ActivationFunctionType.Copy)
                        g = p * (NQ - NQ32) + (q - NQ32)
                        nc.sync.dma_start(
                            o16_dram[:, g * GROUP:(g + 1) * GROUP], ot[:])

    nc.compile()
    _cache["nc"] = nc
    return nc


def _pack_weights(weight):
    w6 = np.asarray(weight, dtype=np.float64).reshape(C, C, KS, KS, KS, KS)
    # gw[jv, jw, co, ci, du, dh]
    gw = np.einsum("av,bw,oiuvhw->aboiuh", G4, G4, w6)
    T = np.zeros((C, HI, NPL, KS, C, BH), dtype=np.float64)
    for i, (jv, jw) in enumerate(PLANES):
        for dh in range(KS):
            for ho in range(BH):
                T[:, ho + dh, i, :, :, ho] = \
                    gw[jv, jw, :, :, :, dh].transpose(1, 2, 0)
    return np.ascontiguousarray(
        T.reshape(K_IN, NPL * KS * M_OUT)).astype(np.float16)


def _pack_input_core(x_n, u0):
    xpad = np.zeros((C, USLAB, SP, SP, SP), dtype=np.float32)
    u_lo = max(0, u0 - 1)
    u_hi = min(S, u0 + UCORE + 1)
    xpad[:, u_lo - (u0 - 1):u_hi - (u0 - 1), 1:S + 1, 1:S + 1, 1:S + 1] = \
        x_n[:, u_lo:u_hi]
    xw = np.empty((USLAB, C, HI, SP, HB, SP), dtype=np.float32)
    xt = xpad.transpose(1, 0, 2, 3, 4)
    for hi in range(HI):
        xw[:, :, hi] = xt[:, :, :, hi::BH, :][:, :, :, :HB, :]
    # fwd V then fwd W, both F(4,3): v' = 4vb+cv, w' = 4wb+cw
    dV = np.stack([xw[:, :, :, cv:cv + 4 * (VB - 1) + 1:4] for cv in range(6)],
                  axis=3)
    xv = np.einsum("jc,uihcvbw->uihjvbw", BT4.astype(np.float32), dV)
    dW = np.stack([xv[..., cw:cw + 4 * (WB - 1) + 1:4] for cw in range(6)],
                  axis=-2)
    bx = np.einsum("kc,uihjvbcw->uihjkvbw", BT4.astype(np.float32), dW)
    # bx[u', ci, hi, jv, jw, vb, hb, wb] -> plane order t, then (vb,hb,wb)
    bx = bx.reshape(USLAB, K_IN, NJ4, NJ4, VB, HB, WB)
    out = np.empty((K_IN, NQ, USLAB, 4, PLF), dtype=np.float16)
    for i, (jv, jw) in enumerate(PLANES):
        out[:, i // 4, :, i % 4] = bx[:, :, jv, jw].reshape(
            USLAB, K_IN, PLF).transpose(1, 0, 2)
    return np.ascontiguousarray(out.reshape(K_IN, USLAB * SLABF))


def make_in_maps(inputs):
    x = np.asarray(inputs["inputs"], dtype=np.float32)
    w_packed = _pack_weights(inputs["weight"])
    in_maps = []
    for c in range(N_CORES):
        n, u0 = c // 4, (c % 4) * UCORE
        in_maps.append({"x": _pack_input_core(x[n], u0), "w": w_packed})
    return in_maps


def host_finish(results16, results32, bias):
    b = np.asarray(bias, dtype=np.float32).reshape(C)
    at4 = AT4.astype(np.float32)
    full = np.empty((2, C, S, S, S, S), dtype=np.float32)
    for c in range(N_CORES):
        n, u0 = c // 4, (c % 4) * UCORE
        # reassemble domain [co, ho, plane, u, vb, hb, wb]
        d32 = np.asarray(results32[c]).astype(np.float32).reshape(
            M_OUT, NUP, NQ32 * 4, 2, PLF)
        d16 = np.asarray(results16[c]).astype(np.float32).reshape(
            M_OUT, NUP, (NQ - NQ32) * 4, 2, PLF)
        dcat = np.concatenate([d32, d16], axis=2)   # [96, p, t, e, 144]
        dom = np.empty((C, BH, NJ4, NJ4, UCORE, VB, HB, WB), dtype=np.float32)
        dv = dcat.reshape(C, BH, NUP, NPL, 2, VB, HB, WB)
        for i, (jv, jw) in enumerate(PLANES):
            # u = 2p + e
            dom[:, :, jv, jw] = dv[:, :, :, i].reshape(
                C, BH, NUP * 2, VB, HB, WB)
        y = np.einsum("aj,bk,ohjkuvxw->ouxvahwb", at4, at4, dom)
        # y[co, u, hb, vb, a4, ho, wb, b4]
        y = y.reshape(C, UCORE, HB, VB, 4, BH, WB, 4)
        y = y.transpose(0, 1, 3, 4, 2, 5, 6, 7).reshape(C, UCORE, S, S, S)
        full[n, :, u0:u0 + UCORE] = y + b[:, None, None, None, None]
    return full


def kernel(inputs, weight, bias):
    nc = _build_nc()
    in_maps = make_in_maps({"inputs": inputs, "weight": weight})
    res = run_bass_kernel_spmd(nc, in_maps, core_ids=list(range(N_CORES)))
    return host_finish([res.results[c]["o16"] for c in range(N_CORES)],
                       [res.results[c]["o32"] for c in range(N_CORES)],
                       bias)


# revision 4
# speedup vs baseline: 1.1157x; 1.0272x over previous
"""Conv4d: F(4,3)^2 Winograd on (V,W), host transforms + Toeplitz-H GEMM.

Per core (8 cores = batch2 x U/4, 6 output-u each):
  - Host: pad, H-window pack (K = 16ci x 8hi = 128), F(4,3) B^T along BOTH
    V and W (6x6 domain planes, 6 vb x 6 wb blocks of 4x4 outputs), cast
    fp16. Device receives the fully transformed tensor:
      bx[(ci,hi)][slab u' (8), plane t (36), (vb6 hb4 wb6)=144]
    packed plane-contiguous so a matmul rhs is [p, (2 slabs), (144)].
  - TensorE: u-pairs. For (p, q, jj, du): one matmul N=288 covers u=2p and
    u=2p+1 (slabs 2p+du, 2p+du+1 stacked in the free dim), K=128, M=96,
    PSUM-accumulate 3 du taps into a [96, 4x512] bank-aligned tile.
    324 matmuls total (93312 rows streamed).
  - ScalarE drains each quad; planes with a high-|A^T| dimension
    (jv or jw in {3,4}) drain to fp32 (20 of 36), the rest to fp16.
  - Host: inverse A^T x A^T, bias, unshuffle.
"""

import sys

if "/opt/trn_rl_repo" not in sys.path:
    sys.path.insert(0, "/opt/trn_rl_repo")

import numpy as np

import concourse.bass as bass
import concourse.mybir as mybir
import concourse.tile as tile
from concourse import bacc
from concourse.bass_utils import run_bass_kernel_spmd

C = 16
KS = 3
S = 24
SP = S + 2
UCORE = 6
USLAB = UCORE + 2
HB = 4               # h blocks
BH = 6               # h outputs per block
HI = 8               # h window size
VB = 6               # v blocks (F(4,3): 4 outputs each)
WB = 6               # w blocks
NJ4 = 6              # F(4,3) domain size per dim
NPL = NJ4 * NJ4      # 36 planes
K_IN = C * HI        # 128
M_OUT = C * BH       # 96
N_CORES = 8
PLF = VB * HB * WB           # 144 cols per (slab, plane)
SLABF = NPL * PLF            # 5184 per slab
NN = 2 * PLF                 # 288 matmul free size (u-pair)
GROUP = 4 * NN               # 1152
NUP = UCORE // 2             # 3 u-pairs
NQ = NPL // 4                # 9 quads
NGRP = NUP * NQ              # 27 groups
OUT_FREE = NGRP * GROUP      # 31104

BT4 = np.array([
    [4, 0, -5, 0, 1, 0],
    [0, -4, -4, 1, 1, 0],
    [0, 4, -4, -1, 1, 0],
    [0, -2, -1, 2, 1, 0],
    [0, 2, -1, -2, 1, 0],
    [0, 4, 0, -5, 0, 1]], dtype=np.float64)
G4 = np.array([
    [1 / 4, 0, 0],
    [-1 / 6, -1 / 6, -1 / 6],
    [-1 / 6, 1 / 6, -1 / 6],
    [1 / 24, 1 / 12, 1 / 6],
    [1 / 24, -1 / 12, 1 / 6],
    [0, 0, 1]], dtype=np.float64)
AT4 = np.array([
    [1, 1, 1, 1, 1, 0],
    [0, 1, -1, 2, -2, 0],
    [0, 1, 1, 4, 4, 0],
    [0, 1, -1, 8, -8, 1]], dtype=np.float64)

# plane order: the 4 worst-amplification planes (both jv and jw in {3,4})
# first — they drain to fp32 and their heavier output DMA should not land
# in the tail — then the rest (fp16 drain)
_HISET = (3, 4)
PLANES = ([(a, b) for a in range(NJ4) for b in range(NJ4)
           if a in _HISET and b in _HISET] +
          [(a, b) for a in range(NJ4) for b in range(NJ4)
           if not (a in _HISET and b in _HISET)])
NQ32 = 1             # quad 0 drains fp32; quads 1-8 drain fp16
OUT32_FREE = NUP * NQ32 * GROUP          # fp32 output columns
OUT16_FREE = NUP * (NQ - NQ32) * GROUP   # fp16 output columns

_cache = {}


def _build_nc():
    if "nc" in _cache:
        return _cache["nc"]
    f16 = mybir.dt.float16
    f32 = mybir.dt.float32
    nc = bacc.Bacc("TRN2", target_bir_lowering=False, debug=False,
                   num_devices=N_CORES)
    x_dram = nc.dram_tensor("x", [K_IN, USLAB * SLABF], f16,
                            kind="ExternalInput")   # (q9, s8, jj4, 144)
    w_dram = nc.dram_tensor("w", [K_IN, NPL * KS * M_OUT], f16,
                            kind="ExternalInput")
    o16_dram = nc.dram_tensor("o16", [M_OUT, OUT16_FREE], f16,
                              kind="ExternalOutput")
    o32_dram = nc.dram_tensor("o32", [M_OUT, OUT32_FREE], f32,
                              kind="ExternalOutput")

    with tile.TileContext(nc) as tc:
        with (
            tc.tile_pool(name="wp", bufs=1) as wp,
            tc.tile_pool(name="bxp", bufs=1) as bxp,
            tc.tile_pool(name="o16p", bufs=24) as o16p,
            tc.tile_pool(name="o32p", bufs=3) as o32p,
            tc.tile_pool(name="ps", bufs=2, space="PSUM") as ps,
        ):
            wt = wp.tile([K_IN, NPL * KS * M_OUT], f16)
            bxt = bxp.tile([K_IN, USLAB * SLABF], f16)

            SPQ = 2 * 4 * PLF        # slab-pair chunk within a quad
            QF = USLAB * 4 * PLF     # 4608 cols per quad

            def dma_chunk(q, sp):
                # slabs (2sp, 2sp+1) of quad q
                c0 = q * QF + sp * SPQ
                nc.sync.dma_start(bxt[:, c0:c0 + SPQ], x_dram[:, c0:c0 + SPQ])

            wcols = NPL * KS * M_OUT // 9    # one weight chunk per quad

            def dma_wchunk(q):
                nc.sync.dma_start(wt[:, q * wcols:(q + 1) * wcols],
                                  w_dram[:, q * wcols:(q + 1) * wcols])

            # q-major arrival: each quad's 4 slab-pair chunks + weight
            # chunk (1.47MB) feed three consecutive (q, p) groups (~5.1us
            # of matmul work) -- 0.29MB/us, under the ~0.42MB/us DMA
            # capacity, so the stream is never feed-starved
            for q in range(NQ):
                dma_chunk(q, 0)
                dma_chunk(q, 1)
                dma_wchunk(q)
                dma_chunk(q, 2)
                dma_chunk(q, 3)

            bv = bxt[:].rearrange("p (q s j f) -> p q s j f",
                                  q=NQ, s=USLAB, j=4)

            for q in range(NQ):
                for p in range(NUP):
                    pt = ps.tile([M_OUT, 4 * 512], f32, tag="acc",
                                 name="acc")
                    for du in range(KS):
                        for jj in range(4):
                            t = q * 4 + jj
                            dst = pt[:, jj * 512:jj * 512 + NN]
                            rhs = bv[:, q, 2 * p + du:2 * p + du + 2, jj]
                            col = (t * KS + du) * M_OUT
                            nc.tensor.matmul(
                                dst,
                                wt[:, col:col + M_OUT],
                                rhs,
                                start=(du == 0),
                                stop=(du == KS - 1),
                            )
                    pv = pt[:].rearrange("p (jj x) -> p jj x", jj=4)
                    if q < NQ32:
                        ot = o32p.tile([M_OUT, GROUP], f32, tag="o32",
                                       name="ot32")
                        ov = ot[:].rearrange("p (jj x) -> p jj x", jj=4)
                        nc.scalar.activation(
                            ov, pv[:, :, 0:NN],
                            mybir.ActivationFunctionType.Copy)
                        g = q * NUP + p
                        nc.sync.dma_start(
                            o32_dram[:, g * GROUP:(g + 1) * GROUP], ot[:])
                    else:
                        ot = o16p.tile([M_OUT, GROUP], f16, tag="o16",
                                       name="ot16")
                        ov = ot[:].rearrange("p (jj x) -> p jj x", jj=4)
                        nc.scalar.activation(
                            ov, pv[:, :, 0:NN],
                            mybir.ActivationFunctionType.Copy)
                        g = (q - NQ32) * NUP + p
                        nc.sync.dma_start(
                            o16_dram[:, g * GROUP:(g + 1) * GROUP], ot[:])

    nc.compile()
    _cache["nc"] = nc
    return nc


def _pack_weights(weight):
    w6 = np.asarray(weight, dtype=np.float64).reshape(C, C, KS, KS, KS, KS)
    # gw[jv, jw, co, ci, du, dh]
    gw = np.einsum("av,bw,oiuvhw->aboiuh", G4, G4, w6)
    T = np.zeros((C, HI, NPL, KS, C, BH), dtype=np.float64)
    for i, (jv, jw) in enumerate(PLANES):
        for dh in range(KS):
            for ho in range(BH):
                T[:, ho + dh, i, :, :, ho] = \
                    gw[jv, jw, :, :, :, dh].transpose(1, 2, 0)
    return np.ascontiguousarray(
        T.reshape(K_IN, NPL * KS * M_OUT)).astype(np.float16)


def _pack_input_core(x_n, u0):
    xpad = np.zeros((C, USLAB, SP, SP, SP), dtype=np.float32)
    u_lo = max(0, u0 - 1)
    u_hi = min(S, u0 + UCORE + 1)
    xpad[:, u_lo - (u0 - 1):u_hi - (u0 - 1), 1:S + 1, 1:S + 1, 1:S + 1] = \
        x_n[:, u_lo:u_hi]
    xw = np.empty((USLAB, C, HI, SP, HB, SP), dtype=np.float32)
    xt = xpad.transpose(1, 0, 2, 3, 4)
    for hi in range(HI):
        xw[:, :, hi] = xt[:, :, :, hi::BH, :][:, :, :, :HB, :]
    # fwd V then fwd W, both F(4,3): v' = 4vb+cv, w' = 4wb+cw
    dV = np.stack([xw[:, :, :, cv:cv + 4 * (VB - 1) + 1:4] for cv in range(6)],
                  axis=3)
    xv = np.einsum("jc,uihcvbw->uihjvbw", BT4.astype(np.float32), dV)
    dW = np.stack([xv[..., cw:cw + 4 * (WB - 1) + 1:4] for cw in range(6)],
                  axis=-2)
    bx = np.einsum("kc,uihjvbcw->uihjkvbw", BT4.astype(np.float32), dW)
    # bx[u', ci, hi, jv, jw, vb, hb, wb] -> plane order t, then (vb,hb,wb)
    bx = bx.reshape(USLAB, K_IN, NJ4, NJ4, VB, HB, WB)
    out = np.empty((K_IN, NQ, USLAB, 4, PLF), dtype=np.float16)
    for i, (jv, jw) in enumerate(PLANES):
        out[:, i // 4, :, i % 4] = bx[:, :, jv, jw].reshape(
            USLAB, K_IN, PLF).transpose(1, 0, 2)
    return np.ascontiguousarray(out.reshape(K_IN, USLAB * SLABF))


def make_in_maps(inputs):
    x = np.asarray(inputs["inputs"], dtype=np.float32)
    w_packed = _pack_weights(inputs["weight"])
    in_maps = []
    for c in range(N_CORES):
        n, u0 = c // 4, (c % 4) * UCORE
        in_maps.append({"x": _pack_input_core(x[n], u0), "w": w_packed})
    return in_maps


def host_finish(results16, results32, bias):
    b = np.asarray(bias, dtype=np.float32).reshape(C)
    at4 = AT4.astype(np.float32)
    full = np.empty((2, C, S, S, S, S), dtype=np.float32)
    for c in range(N_CORES):
        n, u0 = c // 4, (c % 4) * UCORE
        # reassemble domain [co, ho, plane, u, vb, hb, wb]
        d32 = np.asarray(results32[c]).astype(np.float32).reshape(
            M_OUT, NQ32, NUP, 4, 2, PLF).transpose(
            0, 2, 1, 3, 4, 5).reshape(M_OUT, NUP, NQ32 * 4, 2, PLF)
        d16 = np.asarray(results16[c]).astype(np.float32).reshape(
            M_OUT, NQ - NQ32, NUP, 4, 2, PLF).transpose(
            0, 2, 1, 3, 4, 5).reshape(M_OUT, NUP, (NQ - NQ32) * 4, 2, PLF)
        dcat = np.concatenate([d32, d16], axis=2)   # [96, p, t, e, 144]
        dom = np.empty((C, BH, NJ4, NJ4, UCORE, VB, HB, WB), dtype=np.float32)
        dv = dcat.reshape(C, BH, NUP, NPL, 2, VB, HB, WB)
        for i, (jv, jw) in enumerate(PLANES):
            # u = 2p + e
            dom[:, :, jv, jw] = dv[:, :, :, i].reshape(
                C, BH, NUP * 2, VB, HB, WB)
        y = np.einsum("aj,bk,ohjkuvxw->ouxvahwb", at4, at4, dom)
        # y[co, u, hb, vb, a4, ho, wb, b4]
        y = y.reshape(C, UCORE, HB, VB, 4, BH, WB, 4)
        y = y.transpose(0, 1, 3, 4, 2, 5, 6, 7).reshape(C, UCORE, S, S, S)
        full[n, :, u0:u0 + UCORE] = y + b[:, None, None, None, None]
    return full


def kernel(inputs, weight, bias):
    nc = _build_nc()
    in_maps = make_in_maps({"inputs": inputs, "weight": weight})
    res = run_bass_kernel_spmd(nc, in_maps, core_ids=list(range(N_CORES)))
    return host_finish([res.results[c]["o16"] for c in range(N_CORES)],
                       [res.results[c]["o32"] for c in range(N_CORES)],
                       bias)


# revision 5
# speedup vs baseline: 1.1344x; 1.0167x over previous
"""Conv4d: F(4,3)^2 Winograd on (V,W), host transforms + Toeplitz-H GEMM.

Per core (8 cores = batch2 x U/4, 6 output-u each):
  - Host: pad, H-window pack (K = 16ci x 8hi = 128), F(4,3) B^T along BOTH
    V and W (6x6 domain planes, 6 vb x 6 wb blocks of 4x4 outputs), cast
    fp16. Device receives the fully transformed tensor:
      bx[(ci,hi)][slab u' (8), plane t (36), (vb6 hb4 wb6)=144]
    packed plane-contiguous so a matmul rhs is [p, (2 slabs), (144)].
  - TensorE: u-pairs. For (p, q, jj, du): one matmul N=288 covers u=2p and
    u=2p+1 (slabs 2p+du, 2p+du+1 stacked in the free dim), K=128, M=96,
    PSUM-accumulate 3 du taps into a [96, 4x512] bank-aligned tile.
    324 matmuls total (93312 rows streamed).
  - ScalarE drains each quad; planes with a high-|A^T| dimension
    (jv or jw in {3,4}) drain to fp32 (20 of 36), the rest to fp16.
  - Host: inverse A^T x A^T, bias, unshuffle.
"""

import sys

if "/opt/trn_rl_repo" not in sys.path:
    sys.path.insert(0, "/opt/trn_rl_repo")

import numpy as np

import concourse.bass as bass
import concourse.mybir as mybir
import concourse.tile as tile
from concourse import bacc
from concourse.bass_utils import run_bass_kernel_spmd

C = 16
KS = 3
S = 24
SP = S + 2
UCORE = 6
USLAB = UCORE + 2
HB = 4               # h blocks
BH = 6               # h outputs per block
HI = 8               # h window size
VB = 6               # v blocks (F(4,3): 4 outputs each)
WB = 6               # w blocks
NJ4 = 6              # F(4,3) domain size per dim
NPL = NJ4 * NJ4      # 36 planes
K_IN = C * HI        # 128
M_OUT = C * BH       # 96
N_CORES = 8
PLF = VB * HB * WB           # 144 cols per (slab, plane)
SLABF = NPL * PLF            # 5184 per slab
NN = 2 * PLF                 # 288 matmul free size (u-pair)
GROUP = 4 * NN               # 1152
NUP = UCORE // 2             # 3 u-pairs
NQ = NPL // 4                # 9 quads
NGRP = NUP * NQ              # 27 groups
OUT_FREE = NGRP * GROUP      # 31104

BT4 = np.array([
    [4, 0, -5, 0, 1, 0],
    [0, -4, -4, 1, 1, 0],
    [0, 4, -4, -1, 1, 0],
    [0, -2, -1, 2, 1, 0],
    [0, 2, -1, -2, 1, 0],
    [0, 4, 0, -5, 0, 1]], dtype=np.float64)
G4 = np.array([
    [1 / 4, 0, 0],
    [-1 / 6, -1 / 6, -1 / 6],
    [-1 / 6, 1 / 6, -1 / 6],
    [1 / 24, 1 / 12, 1 / 6],
    [1 / 24, -1 / 12, 1 / 6],
    [0, 0, 1]], dtype=np.float64)
AT4 = np.array([
    [1, 1, 1, 1, 1, 0],
    [0, 1, -1, 2, -2, 0],
    [0, 1, 1, 4, 4, 0],
    [0, 1, -1, 8, -8, 1]], dtype=np.float64)

# plane order: the 4 worst-amplification planes (both jv and jw in {3,4})
# first — they drain to fp32 and their heavier output DMA should not land
# in the tail — then the rest (fp16 drain)
_HISET = (3, 4)
PLANES = ([(a, b) for a in range(NJ4) for b in range(NJ4)
           if a in _HISET and b in _HISET] +
          [(a, b) for a in range(NJ4) for b in range(NJ4)
           if not (a in _HISET and b in _HISET)])
NQ32 = 0             # all quads drain fp16
OUT32_FREE = NUP * NQ32 * GROUP          # fp32 output columns
OUT16_FREE = NUP * (NQ - NQ32) * GROUP   # fp16 output columns

_cache = {}


def _build_nc():
    if "nc" in _cache:
        return _cache["nc"]
    f16 = mybir.dt.float16
    f32 = mybir.dt.float32
    nc = bacc.Bacc("TRN2", target_bir_lowering=False, debug=False,
                   num_devices=N_CORES)
    x_dram = nc.dram_tensor("x", [K_IN, USLAB * SLABF], f16,
                            kind="ExternalInput")   # (q9, s8, jj4, 144)
    w_dram = nc.dram_tensor("w", [K_IN, NPL * KS * M_OUT], f16,
                            kind="ExternalInput")
    o16_dram = nc.dram_tensor("o16", [M_OUT, OUT16_FREE], f16,
                              kind="ExternalOutput")
    o32_dram = (nc.dram_tensor("o32", [M_OUT, OUT32_FREE], f32,
                              kind="ExternalOutput")
                if OUT32_FREE else None)

    with tile.TileContext(nc) as tc:
        with (
            tc.tile_pool(name="wp", bufs=1) as wp,
            tc.tile_pool(name="bxp", bufs=1) as bxp,
            tc.tile_pool(name="o16p", bufs=24) as o16p,
            tc.tile_pool(name="o32p", bufs=3) as o32p,
            tc.tile_pool(name="ps", bufs=2, space="PSUM") as ps,
        ):
            wt = wp.tile([K_IN, NPL * KS * M_OUT], f16)
            bxt = bxp.tile([K_IN, USLAB * SLABF], f16)

            SPQ = 2 * 4 * PLF        # slab-pair chunk within a quad
            QF = USLAB * 4 * PLF     # 4608 cols per quad

            def dma_chunk(q, sp):
                # slabs (2sp, 2sp+1) of quad q
                c0 = q * QF + sp * SPQ
                nc.sync.dma_start(bxt[:, c0:c0 + SPQ], x_dram[:, c0:c0 + SPQ])

            wcols = NPL * KS * M_OUT // 9    # one weight chunk per quad

            def dma_wchunk(q):
                nc.sync.dma_start(wt[:, q * wcols:(q + 1) * wcols],
                                  w_dram[:, q * wcols:(q + 1) * wcols])

            # q-major arrival: each quad's 4 slab-pair chunks + weight
            # chunk (1.47MB) feed three consecutive (q, p) groups (~5.1us
            # of matmul work) -- 0.29MB/us, under the ~0.42MB/us DMA
            # capacity, so the stream is never feed-starved
            for q in range(NQ):
                dma_chunk(q, 0)
                dma_chunk(q, 1)
                dma_wchunk(q)
                dma_chunk(q, 2)
                dma_chunk(q, 3)

            bv = bxt[:].rearrange("p (q s j f) -> p q s j f",
                                  q=NQ, s=USLAB, j=4)

            for q in range(NQ):
                for p in range(NUP):
                    pt = ps.tile([M_OUT, 4 * 512], f32, tag="acc",
                                 name="acc")
                    for du in range(KS):
                        for jj in range(4):
                            t = q * 4 + jj
                            dst = pt[:, jj * 512:jj * 512 + NN]
                            rhs = bv[:, q, 2 * p + du:2 * p + du + 2, jj]
                            col = (t * KS + du) * M_OUT
                            nc.tensor.matmul(
                                dst,
                                wt[:, col:col + M_OUT],
                                rhs,
                                start=(du == 0),
                                stop=(du == KS - 1),
                            )
                    pv = pt[:].rearrange("p (jj x) -> p jj x", jj=4)
                    if q < NQ32:
                        ot = o32p.tile([M_OUT, GROUP], f32, tag="o32",
                                       name="ot32")
                        ov = ot[:].rearrange("p (jj x) -> p jj x", jj=4)
                        nc.scalar.activation(
                            ov, pv[:, :, 0:NN],
                            mybir.ActivationFunctionType.Copy)
                        g = q * NUP + p
                        nc.sync.dma_start(
                            o32_dram[:, g * GROUP:(g + 1) * GROUP], ot[:])
                    else:
                        ot = o16p.tile([M_OUT, GROUP], f16, tag="o16",
                                       name="ot16")
                        ov = ot[:].rearrange("p (jj x) -> p jj x", jj=4)
                        nc.scalar.activation(
                            ov, pv[:, :, 0:NN],
                            mybir.ActivationFunctionType.Copy)
                        g = (q - NQ32) * NUP + p
                        nc.sync.dma_start(
                            o16_dram[:, g * GROUP:(g + 1) * GROUP], ot[:])

    nc.compile()
    _cache["nc"] = nc
    return nc


def _pack_weights(weight):
    w6 = np.asarray(weight, dtype=np.float64).reshape(C, C, KS, KS, KS, KS)
    # gw[jv, jw, co, ci, du, dh]
    gw = np.einsum("av,bw,oiuvhw->aboiuh", G4, G4, w6)
    T = np.zeros((C, HI, NPL, KS, C, BH), dtype=np.float64)
    for i, (jv, jw) in enumerate(PLANES):
        for dh in range(KS):
            for ho in range(BH):
                T[:, ho + dh, i, :, :, ho] = \
                    gw[jv, jw, :, :, :, dh].transpose(1, 2, 0)
    return np.ascontiguousarray(
        T.reshape(K_IN, NPL * KS * M_OUT)).astype(np.float16)


def _pack_input_core(x_n, u0):
    xpad = np.zeros((C, USLAB, SP, SP, SP), dtype=np.float32)
    u_lo = max(0, u0 - 1)
    u_hi = min(S, u0 + UCORE + 1)
    xpad[:, u_lo - (u0 - 1):u_hi - (u0 - 1), 1:S + 1, 1:S + 1, 1:S + 1] = \
        x_n[:, u_lo:u_hi]
    xw = np.empty((USLAB, C, HI, SP, HB, SP), dtype=np.float32)
    xt = xpad.transpose(1, 0, 2, 3, 4)
    for hi in range(HI):
        xw[:, :, hi] = xt[:, :, :, hi::BH, :][:, :, :, :HB, :]
    # fwd V then fwd W, both F(4,3): v' = 4vb+cv, w' = 4wb+cw
    dV = np.stack([xw[:, :, :, cv:cv + 4 * (VB - 1) + 1:4] for cv in range(6)],
                  axis=3)
    xv = np.einsum("jc,uihcvbw->uihjvbw", BT4.astype(np.float32), dV)
    dW = np.stack([xv[..., cw:cw + 4 * (WB - 1) + 1:4] for cw in range(6)],
                  axis=-2)
    bx = np.einsum("kc,uihjvbcw->uihjkvbw", BT4.astype(np.float32), dW)
    # bx[u', ci, hi, jv, jw, vb, hb, wb] -> plane order t, then (vb,hb,wb)
    bx = bx.reshape(USLAB, K_IN, NJ4, NJ4, VB, HB, WB)
    out = np.empty((K_IN, NQ, USLAB, 4, PLF), dtype=np.float16)
    for i, (jv, jw) in enumerate(PLANES):
        out[:, i // 4, :, i % 4] = bx[:, :, jv, jw].reshape(
            USLAB, K_IN, PLF).transpose(1, 0, 2)
    return np.ascontiguousarray(out.reshape(K_IN, USLAB * SLABF))


def make_in_maps(inputs):
    x = np.asarray(inputs["inputs"], dtype=np.float32)
    w_packed = _pack_weights(inputs["weight"])
    in_maps = []
    for c in range(N_CORES):
        n, u0 = c // 4, (c % 4) * UCORE
        in_maps.append({"x": _pack_input_core(x[n], u0), "w": w_packed})
    return in_maps


def host_finish(results16, results32, bias):
    b = np.asarray(bias, dtype=np.float32).reshape(C)
    at4 = AT4.astype(np.float32)
    full = np.empty((2, C, S, S, S, S), dtype=np.float32)
    for c in range(N_CORES):
        n, u0 = c // 4, (c % 4) * UCORE
        # reassemble domain [co, ho, plane, u, vb, hb, wb]
        d32 = (np.asarray(results32[c]).astype(np.float32).reshape(
            M_OUT, NQ32, NUP, 4, 2, PLF).transpose(
            0, 2, 1, 3, 4, 5).reshape(M_OUT, NUP, NQ32 * 4, 2, PLF)
            if NQ32 else np.zeros((M_OUT, NUP, 0, 2, PLF), np.float32))
        d16 = np.asarray(results16[c]).astype(np.float32).reshape(
            M_OUT, NQ - NQ32, NUP, 4, 2, PLF).transpose(
            0, 2, 1, 3, 4, 5).reshape(M_OUT, NUP, (NQ - NQ32) * 4, 2, PLF)
        dcat = np.concatenate([d32, d16], axis=2)   # [96, p, t, e, 144]
        dom = np.empty((C, BH, NJ4, NJ4, UCORE, VB, HB, WB), dtype=np.float32)
        dv = dcat.reshape(C, BH, NUP, NPL, 2, VB, HB, WB)
        for i, (jv, jw) in enumerate(PLANES):
            # u = 2p + e
            dom[:, :, jv, jw] = dv[:, :, :, i].reshape(
                C, BH, NUP * 2, VB, HB, WB)
        y = np.einsum("aj,bk,ohjkuvxw->ouxvahwb", at4, at4, dom)
        # y[co, u, hb, vb, a4, ho, wb, b4]
        y = y.reshape(C, UCORE, HB, VB, 4, BH, WB, 4)
        y = y.transpose(0, 1, 3, 4, 2, 5, 6, 7).reshape(C, UCORE, S, S, S)
        full[n, :, u0:u0 + UCORE] = y + b[:, None, None, None, None]
    return full


def kernel(inputs, weight, bias):
    nc = _build_nc()
    in_maps = make_in_maps({"inputs": inputs, "weight": weight})
    res = run_bass_kernel_spmd(nc, in_maps, core_ids=list(range(N_CORES)))
    return host_finish([res.results[c]["o16"] for c in range(N_CORES)],
                       [res.results[c].get("o32") for c in range(N_CORES)],
                       bias)


# revision 6
# speedup vs baseline: 1.1564x; 1.0195x over previous
"""Conv4d: F(4,3)^2 Winograd on (V,W), host transforms + Toeplitz-H GEMM.

Per core (8 cores = batch2 x U/4, 6 output-u each):
  - Host: pad, H-window pack (K = 16ci x 8hi = 128), F(4,3) B^T along BOTH
    V and W (6x6 domain planes, 6 vb x 6 wb blocks of 4x4 outputs), cast
    fp16. Device receives the fully transformed tensor:
      bx[(ci,hi)][slab u' (8), plane t (36), (vb6 hb4 wb6)=144]
    packed plane-contiguous so a matmul rhs is [p, (2 slabs), (144)].
  - TensorE: u-pairs. For (p, q, jj, du): one matmul N=288 covers u=2p and
    u=2p+1 (slabs 2p+du, 2p+du+1 stacked in the free dim), K=128, M=96,
    PSUM-accumulate 3 du taps into a [96, 4x512] bank-aligned tile.
    324 matmuls total (93312 rows streamed).
  - ScalarE drains each quad; planes with a high-|A^T| dimension
    (jv or jw in {3,4}) drain to fp32 (20 of 36), the rest to fp16.
  - Host: inverse A^T x A^T, bias, unshuffle.
"""

import sys

if "/opt/trn_rl_repo" not in sys.path:
    sys.path.insert(0, "/opt/trn_rl_repo")

import numpy as np

import concourse.bass as bass
import concourse.mybir as mybir
import concourse.tile as tile
from concourse import bacc
from concourse.bass_utils import run_bass_kernel_spmd

C = 16
KS = 3
S = 24
SP = S + 2
UCORE = 6
USLAB = UCORE + 2
HB = 4               # h blocks
BH = 6               # h outputs per block
HI = 8               # h window size
VB = 6               # v blocks (F(4,3): 4 outputs each)
WB = 6               # w blocks
NJ4 = 6              # F(4,3) domain size per dim
NPL = NJ4 * NJ4      # 36 planes
K_IN = C * HI        # 128
M_OUT = C * BH       # 96
N_CORES = 8
PLF = VB * HB * WB           # 144 cols per (slab, plane)
SLABF = NPL * PLF            # 5184 per slab
NN = 2 * PLF                 # 288 matmul free size (u-pair)
GROUP = 4 * NN               # 1152
NUP = UCORE // 2             # 3 u-pairs
NQ = NPL // 4                # 9 quads
NGRP = NUP * NQ              # 27 groups
OUT_FREE = NGRP * GROUP      # 31104

BT4 = np.array([
    [4, 0, -5, 0, 1, 0],
    [0, -4, -4, 1, 1, 0],
    [0, 4, -4, -1, 1, 0],
    [0, -2, -1, 2, 1, 0],
    [0, 2, -1, -2, 1, 0],
    [0, 4, 0, -5, 0, 1]], dtype=np.float64)
G4 = np.array([
    [1 / 4, 0, 0],
    [-1 / 6, -1 / 6, -1 / 6],
    [-1 / 6, 1 / 6, -1 / 6],
    [1 / 24, 1 / 12, 1 / 6],
    [1 / 24, -1 / 12, 1 / 6],
    [0, 0, 1]], dtype=np.float64)
AT4 = np.array([
    [1, 1, 1, 1, 1, 0],
    [0, 1, -1, 2, -2, 0],
    [0, 1, 1, 4, 4, 0],
    [0, 1, -1, 8, -8, 1]], dtype=np.float64)

# plane order: the 4 worst-amplification planes (both jv and jw in {3,4})
# first — they drain to fp32 and their heavier output DMA should not land
# in the tail — then the rest (fp16 drain)
_HISET = (3, 4)
PLANES = ([(a, b) for a in range(NJ4) for b in range(NJ4)
           if a in _HISET and b in _HISET] +
          [(a, b) for a in range(NJ4) for b in range(NJ4)
           if not (a in _HISET and b in _HISET)])
NQ32 = 0             # all quads drain fp16
OUT32_FREE = NUP * NQ32 * GROUP          # fp32 output columns
OUT16_FREE = NUP * (NQ - NQ32) * GROUP   # fp16 output columns

_cache = {}


def _build_nc():
    if "nc" in _cache:
        return _cache["nc"]
    f16 = mybir.dt.float16
    f32 = mybir.dt.float32
    nc = bacc.Bacc("TRN2", target_bir_lowering=False, debug=False,
                   num_devices=N_CORES)
    x_dram = nc.dram_tensor("x", [K_IN, USLAB * SLABF], f16,
                            kind="ExternalInput")   # (q9, s8, jj4, 144)
    w_dram = nc.dram_tensor("w", [K_IN, NPL * KS * M_OUT], f16,
                            kind="ExternalInput")
    o16_dram = nc.dram_tensor("o16", [M_OUT, OUT16_FREE], f16,
                              kind="ExternalOutput")
    o32_dram = (nc.dram_tensor("o32", [M_OUT, OUT32_FREE], f32,
                              kind="ExternalOutput")
                if OUT32_FREE else None)

    with tile.TileContext(nc) as tc:
        with (
            tc.tile_pool(name="wp", bufs=1) as wp,
            tc.tile_pool(name="bxp", bufs=1) as bxp,
            tc.tile_pool(name="o16p", bufs=24) as o16p,
            tc.tile_pool(name="o32p", bufs=3) as o32p,
            tc.tile_pool(name="ps", bufs=2, space="PSUM") as ps,
        ):
            wt = wp.tile([K_IN, NPL * KS * M_OUT], f16)
            bxt = bxp.tile([K_IN, USLAB * SLABF], f16)

            SPQ = 2 * 4 * PLF        # slab-pair block within a quad
            QF = USLAB * 4 * PLF     # 4608 cols per quad

            def dma_half(q, h):
                # slabs (4h .. 4h+3) of quad q in one transfer: fewer Sync
                # triggers (27 upfront vs 45) lets output-DMA triggers reach
                # the in-order Sync engine ~11us sooner
                c0 = q * QF + h * 2 * SPQ
                nc.sync.dma_start(bxt[:, c0:c0 + 2 * SPQ],
                                  x_dram[:, c0:c0 + 2 * SPQ])

            wcols = NPL * KS * M_OUT // 9    # one weight chunk per quad

            def dma_wchunk(q):
                nc.sync.dma_start(wt[:, q * wcols:(q + 1) * wcols],
                                  w_dram[:, q * wcols:(q + 1) * wcols])

            # q-major arrival: each quad's chunks + weight chunk
            # (1.47MB) feed three consecutive (q, p) groups (~5.1us of
            # matmul work) -- under the ~0.42MB/us DMA capacity, so the
            # stream is never feed-starved; all input is front-loaded
            for q in range(NQ):
                dma_half(q, 0)
                dma_wchunk(q)
                dma_half(q, 1)

            bv = bxt[:].rearrange("p (q s j f) -> p q s j f",
                                  q=NQ, s=USLAB, j=4)

            for q in range(NQ):
                for p in range(NUP):
                    pt = ps.tile([M_OUT, 4 * 512], f32, tag="acc",
                                 name="acc")
                    for du in range(KS):
                        for jj in range(4):
                            t = q * 4 + jj
                            dst = pt[:, jj * 512:jj * 512 + NN]
                            rhs = bv[:, q, 2 * p + du:2 * p + du + 2, jj]
                            col = (t * KS + du) * M_OUT
                            nc.tensor.matmul(
                                dst,
                                wt[:, col:col + M_OUT],
                                rhs,
                                start=(du == 0),
                                stop=(du == KS - 1),
                            )
                    pv = pt[:].rearrange("p (jj x) -> p jj x", jj=4)
                    if q < NQ32:
                        ot = o32p.tile([M_OUT, GROUP], f32, tag="o32",
                                       name="ot32")
                        ov = ot[:].rearrange("p (jj x) -> p jj x", jj=4)
                        nc.scalar.activation(
                            ov, pv[:, :, 0:NN],
                            mybir.ActivationFunctionType.Copy)
                        g = q * NUP + p
                        nc.sync.dma_start(
                            o32_dram[:, g * GROUP:(g + 1) * GROUP], ot[:])
                    else:
                        ot = o16p.tile([M_OUT, GROUP], f16, tag="o16",
                                       name="ot16")
                        ov = ot[:].rearrange("p (jj x) -> p jj x", jj=4)
                        nc.scalar.activation(
                            ov, pv[:, :, 0:NN],
                            mybir.ActivationFunctionType.Copy)
                        g = (q - NQ32) * NUP + p
                        nc.sync.dma_start(
                            o16_dram[:, g * GROUP:(g + 1) * GROUP], ot[:])

    nc.compile()
    _cache["nc"] = nc
    return nc


def _pack_weights(weight):
    w6 = np.asarray(weight, dtype=np.float64).reshape(C, C, KS, KS, KS, KS)
    # gw[jv, jw, co, ci, du, dh]
    gw = np.einsum("av,bw,oiuvhw->aboiuh", G4, G4, w6)
    T = np.zeros((C, HI, NPL, KS, C, BH), dtype=np.float64)
    for i, (jv, jw) in enumerate(PLANES):
        for dh in range(KS):
            for ho in range(BH):
                T[:, ho + dh, i, :, :, ho] = \
                    gw[jv, jw, :, :, :, dh].transpose(1, 2, 0)
    return np.ascontiguousarray(
        T.reshape(K_IN, NPL * KS * M_OUT)).astype(np.float16)


def _pack_input_core(x_n, u0):
    xpad = np.zeros((C, USLAB, SP, SP, SP), dtype=np.float32)
    u_lo = max(0, u0 - 1)
    u_hi = min(S, u0 + UCORE + 1)
    xpad[:, u_lo - (u0 - 1):u_hi - (u0 - 1), 1:S + 1, 1:S + 1, 1:S + 1] = \
        x_n[:, u_lo:u_hi]
    xw = np.empty((USLAB, C, HI, SP, HB, SP), dtype=np.float32)
    xt = xpad.transpose(1, 0, 2, 3, 4)
    for hi in range(HI):
        xw[:, :, hi] = xt[:, :, :, hi::BH, :][:, :, :, :HB, :]
    # fwd V then fwd W, both F(4,3): v' = 4vb+cv, w' = 4wb+cw
    dV = np.stack([xw[:, :, :, cv:cv + 4 * (VB - 1) + 1:4] for cv in range(6)],
                  axis=3)
    xv = np.einsum("jc,uihcvbw->uihjvbw", BT4.astype(np.float32), dV)
    dW = np.stack([xv[..., cw:cw + 4 * (WB - 1) + 1:4] for cw in range(6)],
                  axis=-2)
    bx = np.einsum("kc,uihjvbcw->uihjkvbw", BT4.astype(np.float32), dW)
    # bx[u', ci, hi, jv, jw, vb, hb, wb] -> plane order t, then (vb,hb,wb)
    bx = bx.reshape(USLAB, K_IN, NJ4, NJ4, VB, HB, WB)
    out = np.empty((K_IN, NQ, USLAB, 4, PLF), dtype=np.float16)
    for i, (jv, jw) in enumerate(PLANES):
        out[:, i // 4, :, i % 4] = bx[:, :, jv, jw].reshape(
            USLAB, K_IN, PLF).transpose(1, 0, 2)
    return np.ascontiguousarray(out.reshape(K_IN, USLAB * SLABF))


def make_in_maps(inputs):
    x = np.asarray(inputs["inputs"], dtype=np.float32)
    w_packed = _pack_weights(inputs["weight"])
    in_maps = []
    for c in range(N_CORES):
        n, u0 = c // 4, (c % 4) * UCORE
        in_maps.append({"x": _pack_input_core(x[n], u0), "w": w_packed})
    return in_maps


def host_finish(results16, results32, bias):
    b = np.asarray(bias, dtype=np.float32).reshape(C)
    at4 = AT4.astype(np.float32)
    full = np.empty((2, C, S, S, S, S), dtype=np.float32)
    for c in range(N_CORES):
        n, u0 = c // 4, (c % 4) * UCORE
        # reassemble domain [co, ho, plane, u, vb, hb, wb]
        d32 = (np.asarray(results32[c]).astype(np.float32).reshape(
            M_OUT, NQ32, NUP, 4, 2, PLF).transpose(
            0, 2, 1, 3, 4, 5).reshape(M_OUT, NUP, NQ32 * 4, 2, PLF)
            if NQ32 else np.zeros((M_OUT, NUP, 0, 2, PLF), np.float32))
        d16 = np.asarray(results16[c]).astype(np.float32).reshape(
            M_OUT, NQ - NQ32, NUP, 4, 2, PLF).transpose(
            0, 2, 1, 3, 4, 5).reshape(M_OUT, NUP, (NQ - NQ32) * 4, 2, PLF)
        dcat = np.concatenate([d32, d16], axis=2)   # [96, p, t, e, 144]
        dom = np.empty((C, BH, NJ4, NJ4, UCORE, VB, HB, WB), dtype=np.float32)
        dv = dcat.reshape(C, BH, NUP, NPL, 2, VB, HB, WB)
        for i, (jv, jw) in enumerate(PLANES):
            # u = 2p + e
            dom[:, :, jv, jw] = dv[:, :, :, i].reshape(
                C, BH, NUP * 2, VB, HB, WB)
        y = np.einsum("aj,bk,ohjkuvxw->ouxvahwb", at4, at4, dom)
        # y[co, u, hb, vb, a4, ho, wb, b4]
        y = y.reshape(C, UCORE, HB, VB, 4, BH, WB, 4)
        y = y.transpose(0, 1, 3, 4, 2, 5, 6, 7).reshape(C, UCORE, S, S, S)
        full[n, :, u0:u0 + UCORE] = y + b[:, None, None, None, None]
    return full


def kernel(inputs, weight, bias):
    nc = _build_nc()
    in_maps = make_in_maps({"inputs": inputs, "weight": weight})
    res = run_bass_kernel_spmd(nc, in_maps, core_ids=list(range(N_CORES)))
    return host_finish([res.results[c]["o16"] for c in range(N_CORES)],
                       [res.results[c].get("o32") for c in range(N_CORES)],
                       bias)


# revision 7
# speedup vs baseline: 1.1583x; 1.0016x over previous
"""Conv4d: F(4,3)^2 Winograd on (V,W), host transforms + Toeplitz-H GEMM.

Per core (8 cores = batch2 x U/4, 6 output-u each):
  - Host: pad, H-window pack (K = 16ci x 8hi = 128), F(4,3) B^T along BOTH
    V and W (6x6 domain planes, 6 vb x 6 wb blocks of 4x4 outputs), cast
    fp16. Device receives the fully transformed tensor:
      bx[(ci,hi)][slab u' (8), plane t (36), (vb6 hb4 wb6)=144]
    packed plane-contiguous so a matmul rhs is [p, (2 slabs), (144)].
  - TensorE: u-pairs. For (p, q, jj, du): one matmul N=288 covers u=2p and
    u=2p+1 (slabs 2p+du, 2p+du+1 stacked in the free dim), K=128, M=96,
    PSUM-accumulate 3 du taps into a [96, 4x512] bank-aligned tile.
    324 matmuls total (93312 rows streamed).
  - ScalarE drains each quad; planes with a high-|A^T| dimension
    (jv or jw in {3,4}) drain to fp32 (20 of 36), the rest to fp16.
  - Host: inverse A^T x A^T, bias, unshuffle.
"""

import sys

if "/opt/trn_rl_repo" not in sys.path:
    sys.path.insert(0, "/opt/trn_rl_repo")

import numpy as np

import concourse.bass as bass
import concourse.mybir as mybir
import concourse.tile as tile
from concourse import bacc
from concourse.bass_utils import run_bass_kernel_spmd

C = 16
KS = 3
S = 24
SP = S + 2
UCORE = 6
USLAB = UCORE + 2
HB = 4               # h blocks
BH = 6               # h outputs per block
HI = 8               # h window size
VB = 6               # v blocks (F(4,3): 4 outputs each)
WB = 6               # w blocks
NJ4 = 6              # F(4,3) domain size per dim
NPL = NJ4 * NJ4      # 36 planes
K_IN = C * HI        # 128
M_OUT = C * BH       # 96
N_CORES = 8
PLF = VB * HB * WB           # 144 cols per (slab, plane)
SLABF = NPL * PLF            # 5184 per slab
NN = 2 * PLF                 # 288 matmul free size (u-pair)
GROUP = 4 * NN               # 1152
NUP = UCORE // 2             # 3 u-pairs
NQ = NPL // 4                # 9 quads
NGRP = NUP * NQ              # 27 groups
OUT_FREE = NGRP * GROUP      # 31104

BT4 = np.array([
    [4, 0, -5, 0, 1, 0],
    [0, -4, -4, 1, 1, 0],
    [0, 4, -4, -1, 1, 0],
    [0, -2, -1, 2, 1, 0],
    [0, 2, -1, -2, 1, 0],
    [0, 4, 0, -5, 0, 1]], dtype=np.float64)
G4 = np.array([
    [1 / 4, 0, 0],
    [-1 / 6, -1 / 6, -1 / 6],
    [-1 / 6, 1 / 6, -1 / 6],
    [1 / 24, 1 / 12, 1 / 6],
    [1 / 24, -1 / 12, 1 / 6],
    [0, 0, 1]], dtype=np.float64)
AT4 = np.array([
    [1, 1, 1, 1, 1, 0],
    [0, 1, -1, 2, -2, 0],
    [0, 1, 1, 4, 4, 0],
    [0, 1, -1, 8, -8, 1]], dtype=np.float64)

# plane order: the 4 worst-amplification planes (both jv and jw in {3,4})
# first — they drain to fp32 and their heavier output DMA should not land
# in the tail — then the rest (fp16 drain)
_HISET = (3, 4)
PLANES = ([(a, b) for a in range(NJ4) for b in range(NJ4)
           if a in _HISET and b in _HISET] +
          [(a, b) for a in range(NJ4) for b in range(NJ4)
           if not (a in _HISET and b in _HISET)])
NQ32 = 0             # all quads drain fp16
OUT32_FREE = NUP * NQ32 * GROUP          # fp32 output columns
OUT16_FREE = NUP * (NQ - NQ32) * GROUP   # fp16 output columns

_cache = {}


def _build_nc():
    if "nc" in _cache:
        return _cache["nc"]
    f16 = mybir.dt.float16
    f32 = mybir.dt.float32
    nc = bacc.Bacc("TRN2", target_bir_lowering=False, debug=False,
                   num_devices=N_CORES)
    x_dram = nc.dram_tensor("x", [K_IN, USLAB * SLABF], f16,
                            kind="ExternalInput")   # (q9, s8, jj4, 144)
    w_dram = nc.dram_tensor("w", [K_IN, NPL * KS * M_OUT], f16,
                            kind="ExternalInput")
    o16_dram = nc.dram_tensor("o16", [M_OUT, OUT16_FREE], f16,
                              kind="ExternalOutput")
    o32_dram = (nc.dram_tensor("o32", [M_OUT, OUT32_FREE], f32,
                              kind="ExternalOutput")
                if OUT32_FREE else None)

    with tile.TileContext(nc) as tc:
        with (
            tc.tile_pool(name="wp", bufs=1) as wp,
            tc.tile_pool(name="bxp", bufs=1) as bxp,
            tc.tile_pool(name="o16p", bufs=24) as o16p,
            tc.tile_pool(name="o32p", bufs=3) as o32p,
            tc.tile_pool(name="ps", bufs=2, space="PSUM") as ps,
        ):
            wt = wp.tile([K_IN, NPL * KS * M_OUT], f16)
            bxt = bxp.tile([K_IN, USLAB * SLABF], f16)

            SPQ = 2 * 4 * PLF        # slab-pair block within a quad
            QF = USLAB * 4 * PLF     # 4608 cols per quad

            def dma_half(q, h):
                # slabs (4h .. 4h+3) of quad q in one transfer: fewer Sync
                # triggers (27 upfront vs 45) lets output-DMA triggers reach
                # the in-order Sync engine ~11us sooner
                c0 = q * QF + h * 2 * SPQ
                nc.sync.dma_start(bxt[:, c0:c0 + 2 * SPQ],
                                  x_dram[:, c0:c0 + 2 * SPQ])

            wcols = NPL * KS * M_OUT // 9    # one weight chunk per quad

            def dma_wchunk(q):
                nc.sync.dma_start(wt[:, q * wcols:(q + 1) * wcols],
                                  w_dram[:, q * wcols:(q + 1) * wcols])

            # q-major arrival: each quad's chunks + weight chunk
            # (1.47MB) feed three consecutive (q, p) groups (~5.1us of
            # matmul work) -- under the ~0.42MB/us DMA capacity, so the
            # stream is never feed-starved; all input is front-loaded
            for q in range(NQ):
                dma_half(q, 0)
                dma_wchunk(q)
                dma_half(q, 1)

            bv = bxt[:].rearrange("p (q s j f) -> p q s j f",
                                  q=NQ, s=USLAB, j=4)

            for q in range(NQ):
                for p in range(NUP):
                    pt = ps.tile([M_OUT, 4 * 512], f32, tag="acc",
                                 name="acc")
                    for du in range(KS):
                        for jj in range(4):
                            t = q * 4 + jj
                            dst = pt[:, jj * 512:jj * 512 + NN]
                            rhs = bv[:, q, 2 * p + du:2 * p + du + 2, jj]
                            col = (t * KS + du) * M_OUT
                            nc.tensor.matmul(
                                dst,
                                wt[:, col:col + M_OUT],
                                rhs,
                                start=(du == 0),
                                stop=(du == KS - 1),
                            )
                    pv = pt[:].rearrange("p (jj x) -> p jj x", jj=4)
                    if q < NQ32:
                        ot = o32p.tile([M_OUT, GROUP], f32, tag="o32",
                                       name="ot32")
                        ov = ot[:].rearrange("p (jj x) -> p jj x", jj=4)
                        nc.scalar.activation(
                            ov, pv[:, :, 0:NN],
                            mybir.ActivationFunctionType.Copy)
                        g = q * NUP + p
                        nc.sync.dma_start(
                            o32_dram[:, g * GROUP:(g + 1) * GROUP], ot[:])
                    else:
                        ot = o16p.tile([M_OUT, GROUP], f16, tag="o16",
                                       name="ot16")
                        ov = ot[:].rearrange("p (jj x) -> p jj x", jj=4)
                        # alternate drains between ScalarE and the idle DVE:
                        # the output flush is drain-paced (one group per
                        # ~1.55us on Act vs ~0.7us of queue time), so two
                        # engines make groups DMA-ready twice as fast
                        if (q * NUP + p) % 2 == 0:
                            nc.scalar.activation(
                                ov, pv[:, :, 0:NN],
                                mybir.ActivationFunctionType.Copy)
                        else:
                            nc.vector.tensor_copy(ov, pv[:, :, 0:NN])
                        g = (q - NQ32) * NUP + p
                        nc.sync.dma_start(
                            o16_dram[:, g * GROUP:(g + 1) * GROUP], ot[:])

    nc.compile()
    _cache["nc"] = nc
    return nc


def _pack_weights(weight):
    w6 = np.asarray(weight, dtype=np.float64).reshape(C, C, KS, KS, KS, KS)
    # gw[jv, jw, co, ci, du, dh]
    gw = np.einsum("av,bw,oiuvhw->aboiuh", G4, G4, w6)
    T = np.zeros((C, HI, NPL, KS, C, BH), dtype=np.float64)
    for i, (jv, jw) in enumerate(PLANES):
        for dh in range(KS):
            for ho in range(BH):
                T[:, ho + dh, i, :, :, ho] = \
                    gw[jv, jw, :, :, :, dh].transpose(1, 2, 0)
    return np.ascontiguousarray(
        T.reshape(K_IN, NPL * KS * M_OUT)).astype(np.float16)


def _pack_input_core(x_n, u0):
    xpad = np.zeros((C, USLAB, SP, SP, SP), dtype=np.float32)
    u_lo = max(0, u0 - 1)
    u_hi = min(S, u0 + UCORE + 1)
    xpad[:, u_lo - (u0 - 1):u_hi - (u0 - 1), 1:S + 1, 1:S + 1, 1:S + 1] = \
        x_n[:, u_lo:u_hi]
    xw = np.empty((USLAB, C, HI, SP, HB, SP), dtype=np.float32)
    xt = xpad.transpose(1, 0, 2, 3, 4)
    for hi in range(HI):
        xw[:, :, hi] = xt[:, :, :, hi::BH, :][:, :, :, :HB, :]
    # fwd V then fwd W, both F(4,3): v' = 4vb+cv, w' = 4wb+cw
    dV = np.stack([xw[:, :, :, cv:cv + 4 * (VB - 1) + 1:4] for cv in range(6)],
                  axis=3)
    xv = np.einsum("jc,uihcvbw->uihjvbw", BT4.astype(np.float32), dV)
    dW = np.stack([xv[..., cw:cw + 4 * (WB - 1) + 1:4] for cw in range(6)],
                  axis=-2)
    bx = np.einsum("kc,uihjvbcw->uihjkvbw", BT4.astype(np.float32), dW)
    # bx[u', ci, hi, jv, jw, vb, hb, wb] -> plane order t, then (vb,hb,wb)
    bx = bx.reshape(USLAB, K_IN, NJ4, NJ4, VB, HB, WB)
    out = np.empty((K_IN, NQ, USLAB, 4, PLF), dtype=np.float16)
    for i, (jv, jw) in enumerate(PLANES):
        out[:, i // 4, :, i % 4] = bx[:, :, jv, jw].reshape(
            USLAB, K_IN, PLF).transpose(1, 0, 2)
    return np.ascontiguousarray(out.reshape(K_IN, USLAB * SLABF))


def make_in_maps(inputs):
    x = np.asarray(inputs["inputs"], dtype=np.float32)
    w_packed = _pack_weights(inputs["weight"])
    in_maps = []
    for c in range(N_CORES):
        n, u0 = c // 4, (c % 4) * UCORE
        in_maps.append({"x": _pack_input_core(x[n], u0), "w": w_packed})
    return in_maps


def host_finish(results16, results32, bias):
    b = np.asarray(bias, dtype=np.float32).reshape(C)
    at4 = AT4.astype(np.float32)
    full = np.empty((2, C, S, S, S, S), dtype=np.float32)
    for c in range(N_CORES):
        n, u0 = c // 4, (c % 4) * UCORE
        # reassemble domain [co, ho, plane, u, vb, hb, wb]
        d32 = (np.asarray(results32[c]).astype(np.float32).reshape(
            M_OUT, NQ32, NUP, 4, 2, PLF).transpose(
            0, 2, 1, 3, 4, 5).reshape(M_OUT, NUP, NQ32 * 4, 2, PLF)
            if NQ32 else np.zeros((M_OUT, NUP, 0, 2, PLF), np.float32))
        d16 = np.asarray(results16[c]).astype(np.float32).reshape(
            M_OUT, NQ - NQ32, NUP, 4, 2, PLF).transpose(
            0, 2, 1, 3, 4, 5).reshape(M_OUT, NUP, (NQ - NQ32) * 4, 2, PLF)
        dcat = np.concatenate([d32, d16], axis=2)   # [96, p, t, e, 144]
        dom = np.empty((C, BH, NJ4, NJ4, UCORE, VB, HB, WB), dtype=np.float32)
        dv = dcat.reshape(C, BH, NUP, NPL, 2, VB, HB, WB)
        for i, (jv, jw) in enumerate(PLANES):
            # u = 2p + e
            dom[:, :, jv, jw] = dv[:, :, :, i].reshape(
                C, BH, NUP * 2, VB, HB, WB)
        y = np.einsum("aj,bk,ohjkuvxw->ouxvahwb", at4, at4, dom)
        # y[co, u, hb, vb, a4, ho, wb, b4]
        y = y.reshape(C, UCORE, HB, VB, 4, BH, WB, 4)
        y = y.transpose(0, 1, 3, 4, 2, 5, 6, 7).reshape(C, UCORE, S, S, S)
        full[n, :, u0:u0 + UCORE] = y + b[:, None, None, None, None]
    return full


def kernel(inputs, weight, bias):
    nc = _build_nc()
    in_maps = make_in_maps({"inputs": inputs, "weight": weight})
    res = run_bass_kernel_spmd(nc, in_maps, core_ids=list(range(N_CORES)))
    return host_finish([res.results[c]["o16"] for c in range(N_CORES)],
                       [res.results[c].get("o32") for c in range(N_CORES)],
                       bias)
